# revision 12
# baseline (speedup 1.0000x reference)
"""ClassicalSelfAttention (B=4, N=4096, D=1024, fp32) on 8 Trainium2 NeuronCores.

out[b,n] = (softmax(Q K^T / sqrt(D)) V).mean(-1) = softmax(...) @ vbar,
with vbar = X @ Wv.mean(1)  (the mean commutes with the V projection),
eliminating the V projection and the AV matmul entirely.

Weight fusion: logits/sqrt(D) = X (Wq Wk^T / sqrt(D)) X^T = (X Wqk~) X^T,
so the K projection disappears as well — X^T itself is streamed as the
"key" operand, and Wqk~ is computed on host (a 1024x1024 GEMM).

Sharding: core c -> (batch b=c//2, query-half h=c%2). Per core:
QT~ (2048-query half) SBUF-resident after a single projection pass;
flash-style m-outer attention streaming X^T chunks with per-chunk softmax
stats and a deferred combine. Matmuls run as float32r (fp32, 11-bit
mantissa kept, fp32 PSUM accumulation) at full PE rate.

Engine balance in the attention phase (per 128x512 logits tile):
  PE:  8 accumulating matmuls
  DVE: negated max-reduce (feeds exp bias directly),
       fused (e * vbar) + accumulate via scalar_tensor_tensor
  Act: exp with accum_out (sum of exponentials for free)
"""

import numpy as np

import concourse.bacc as bacc
import concourse.mybir as mybir
import concourse.tile as tile
from concourse.bass_utils import run_bass_kernel_spmd

from contextlib import ExitStack

F32 = mybir.dt.float32
F32R = mybir.dt.float32r
BF16 = mybir.dt.bfloat16

D = 1024
DC = 8  # d chunks of 128
NQ = 2048  # queries per core
QT_N = 16  # q tiles of 128
M = 4096  # keys
MC = 8  # m chunks of 512

Exp = mybir.ActivationFunctionType.Exp
Alu = mybir.AluOpType
AxX = mybir.AxisListType.X


def build(n_cores=8):
    nc = bacc.Bacc("TRN2", target_bir_lowering=False, debug=False,
                   num_devices=n_cores)

    xt_d = nc.dram_tensor("xt", [DC, 128, M], F32R, kind="ExternalInput")
    xtq_d = nc.dram_tensor("xtq", [DC, 128, NQ], F32R, kind="ExternalInput")
    wqk_d = nc.dram_tensor("wqk", [DC, 128, D], F32R, kind="ExternalInput")
    wvb_d = nc.dram_tensor("wvb", [DC, 128, 128], F32R, kind="ExternalInput")
    # [128, 16]: column q holds the outputs of queries q*128..q*128+127;
    # the host transposes after download (single contiguous-line DMA here).
    out_d = nc.dram_tensor("out", [128, QT_N], F32, kind="ExternalOutput")

    with tile.TileContext(nc) as tc, ExitStack() as ctx:
        pw = ctx.enter_context(tc.tile_pool(name="pw", bufs=1))
        px = ctx.enter_context(tc.tile_pool(name="px", bufs=3))
        pqt = ctx.enter_context(tc.tile_pool(name="pqt", bufs=1))
        pvb = ctx.enter_context(tc.tile_pool(name="pvb", bufs=1))
        pe_ = ctx.enter_context(tc.tile_pool(name="pe", bufs=4))
        pj = ctx.enter_context(tc.tile_pool(name="pj", bufs=2))
        pst = ctx.enter_context(tc.tile_pool(name="pst", bufs=1))
        psm = ctx.enter_context(tc.tile_pool(name="psm", bufs=4))
        pps = ctx.enter_context(tc.tile_pool(name="pps", bufs=6, space="PSUM"))

        # ---- resident tiles ----
        qt_sb = [pqt.tile([128, NQ], F32R, name=f"qt{do}", tag=f"qt{do}")
                 for do in range(DC)]
        vbar_bc = pvb.tile([128, MC, 512], BF16, name="vbar", tag="vbar")

        # weights on the scalar-engine HWDGE queue: runs in parallel with the
        # xtq loads on the sync queue, so phase-1 matmuls start ~2x sooner
        wqk_t = [pw.tile([128, D], F32R, name=f"wqk{di}", tag=f"wqk{di}")
                 for di in range(DC)]
        for di in range(DC):
            nc.scalar.dma_start(wqk_t[di][:], wqk_d.ap()[di])
        wvb_t = [pw.tile([128, 128], F32R, name=f"wvb{di}", tag=f"wvb{di}")
                 for di in range(DC)]
        for di in range(DC):
            nc.scalar.dma_start(wvb_t[di][:], wvb_d.ap()[di])

        # ---- phase 1: QT~ = (X_half @ Wqk~)^T, kept SBUF-resident ----
        for n in range(NQ // 512):
            xq_t = [px.tile([128, 512], F32R, name=f"x{di}", tag=f"x{di}")
                    for di in range(DC)]
            for di in range(DC):
                nc.sync.dma_start(xq_t[di][:], xtq_d.ap()[di, :, n * 512:(n + 1) * 512])
            for do in range(DC):
                qtp = pps.tile([128, 512], F32, name="ps", tag="ps")
                for di in range(DC):
                    nc.tensor.matmul(qtp[:], wqk_t[di][:, do * 128:(do + 1) * 128],
                                     xq_t[di][:], start=(di == 0), stop=(di == DC - 1))
                nc.scalar.copy(qt_sb[do][:, n * 512:(n + 1) * 512], qtp[:])

        # ---- phase 2: attention, m-outer ----
        nmst = [pst.tile([128, MC], F32, name=f"m{q}", tag=f"m{q}") for q in range(QT_N)]
        dstk = [pst.tile([128, MC], F32, name=f"d{q}", tag=f"d{q}") for q in range(QT_N)]
        nstk = [pst.tile([128, MC], F32, name=f"n{q}", tag=f"n{q}") for q in range(QT_N)]

        for mi in range(MC):
            kt_t = [px.tile([128, 512], F32R, name=f"x{di}", tag=f"x{di}")
                    for di in range(DC)]
            for di in range(DC):
                nc.sync.dma_start(kt_t[di][:], xt_d.ap()[di, :, mi * 512:(mi + 1) * 512])
            # vbar chunk: all 128 partitions get identical rows
            vbp = pps.tile([128, 512], F32, name="ps", tag="ps")
            for di in range(DC):
                nc.tensor.matmul(vbp[:], wvb_t[di][:], kt_t[di][:],
                                 start=(di == 0), stop=(di == DC - 1))
            nc.vector.tensor_copy(vbar_bc[:, mi, :], vbp[:])

            for q in range(QT_N):
                sp = pps.tile([128, 512], F32, name="ps", tag="ps")
                for di in range(DC):
                    nc.tensor.matmul(sp[:], qt_sb[di][:, q * 128:(q + 1) * 128],
                                     kt_t[di][:], start=(di == 0), stop=(di == DC - 1))
                nc.vector.tensor_reduce(nmst[q][:, mi:mi + 1], sp[:], axis=AxX,
                                        op=Alu.max, negate=True)
                e_t = pe_.tile([128, 512], BF16, name="e", tag="e")
                nc.scalar.activation(e_t[:], sp[:], Exp,
                                     bias=nmst[q][:, mi:mi + 1], scale=1.0,
                                     accum_out=dstk[q][:, mi:mi + 1])
                junk = pj.tile([128, 512], BF16, name="j", tag="j")
                nc.vector.scalar_tensor_tensor(junk[:], e_t[:], 1.0,
                                               vbar_bc[:, mi, :],
                                               op0=Alu.mult, op1=Alu.mult,
                                               accum_out=nstk[q][:, mi:mi + 1])

        # ---- combine + output ----
        o_all = pst.tile([128, QT_N], F32, name="oall", tag="oall")
        for q in range(QT_N):
            gm = psm.tile([128, 1], F32, name="gm", tag="gm")
            nc.vector.tensor_reduce(gm[:], nmst[q][:], axis=AxX, op=Alu.min)
            w8 = psm.tile([128, MC], F32, name="w8", tag="w8")
            nc.scalar.activation(w8[:], nmst[q][:], Exp, bias=gm[:], scale=-1.0)
            tn = psm.tile([128, MC], F32, name="tn", tag="tn")
            num = psm.tile([128, 1], F32, name="num", tag="num")
            nc.vector.scalar_tensor_tensor(tn[:], w8[:], 1.0, nstk[q][:],
                                           op0=Alu.mult, op1=Alu.mult,
                                           accum_out=num[:])
            td = psm.tile([128, MC], F32, name="td", tag="td")
            den = psm.tile([128, 1], F32, name="den", tag="den")
            nc.vector.scalar_tensor_tensor(td[:], w8[:], 1.0, dstk[q][:],
                                           op0=Alu.mult, op1=Alu.mult,
                                           accum_out=den[:])
            rec = psm.tile([128, 1], F32, name="rec", tag="rec")
            nc.vector.reciprocal(rec[:], den[:])
            nc.vector.tensor_tensor(o_all[:, q:q + 1], num[:], rec[:], op=Alu.mult)
        nc.sync.dma_start(out_d.ap()[:], o_all[:])

    nc.compile()
    return nc


def r32r(x):
    """Round fp32 -> fp32r (keep 11 mantissa bits, round-to-nearest-even)."""
    u = np.ascontiguousarray(x, dtype=np.float32).view(np.uint32)
    low = u & np.uint32(0xFFF)
    add = np.where((low > 0x800) | ((low == 0x800) & (((u >> np.uint32(12)) & 1) > 0)),
                   np.uint32(0x1000), np.uint32(0))
    return ((u + add) & np.uint32(0xFFFFF000)).view(np.float32)


def make_in_maps(inputs, Wq, Wk, Wv):
    """inputs [4,4096,1024] f32; weights [1024,1024]. Returns 8 in_maps."""
    B = inputs.shape[0]
    SCALE = np.float32(1.0 / 32.0)
    wqk = (Wq.astype(np.float32) @ Wk.astype(np.float32).T) * SCALE
    wqk_r = r32r(wqk).reshape(DC, 128, D)
    wvbar = (Wv.astype(np.float32).sum(axis=1) * np.float32(1.0 / D))
    wvb_r = np.repeat(r32r(wvbar).reshape(DC, 128, 1), 128, axis=2)
    wvb_r = np.ascontiguousarray(wvb_r)
    in_maps = []
    for c in range(2 * B):
        b, h = divmod(c, 2)
        xt = r32r(np.ascontiguousarray(inputs[b].T))  # [1024, 4096]
        xtq = np.ascontiguousarray(xt[:, h * NQ:(h + 1) * NQ])
        in_maps.append({
            "xt": np.ascontiguousarray(xt.reshape(DC, 128, M)),
            "xtq": np.ascontiguousarray(xtq.reshape(DC, 128, NQ)),
            "wqk": wqk_r, "wvb": wvb_r,
        })
    return in_maps


def assemble(results, B=4):
    out = np.empty((B, M), dtype=np.float32)
    for c in range(2 * B):
        b, h = divmod(c, 2)
        # device emits [128, 16] with column q = queries q*128..q*128+127
        out[b, h * NQ:(h + 1) * NQ] = np.ascontiguousarray(results[c]["out"].T).reshape(NQ)
    return out


_NC_CACHE = {}


def _get_nc():
    if "nc" not in _NC_CACHE:
        _NC_CACHE["nc"] = build(8)
    return _NC_CACHE["nc"]


def kernel(inputs, Wq, Wk, Wv):
    inputs = np.asarray(inputs, dtype=np.float32)
    Wq = np.asarray(Wq, dtype=np.float32)
    Wk = np.asarray(Wk, dtype=np.float32)
    Wv = np.asarray(Wv, dtype=np.float32)
    nc = _get_nc()
    in_maps = make_in_maps(inputs, Wq, Wk, Wv)
    res = run_bass_kernel_spmd(nc, in_maps, core_ids=list(range(8)), trace=False)
    return assemble(res.results, B=inputs.shape[0])


# revision 13
# speedup vs baseline: 1.0166x; 1.0166x over previous
"""ClassicalSelfAttention (B=4, N=4096, D=1024, fp32) on 8 Trainium2 NeuronCores.

out[b,n] = (softmax(Q K^T / sqrt(D)) V).mean(-1) = softmax(...) @ vbar,
with vbar = X @ Wv.mean(1)  (the mean commutes with the V projection),
eliminating the V projection and the AV matmul entirely.

Weight fusion: logits/sqrt(D) = X (Wq Wk^T / sqrt(D)) X^T = (X Wqk~) X^T,
so the K projection disappears as well — X^T itself is streamed as the
"key" operand, and Wqk~ is computed on host (a 1024x1024 GEMM).

Sharding: core c -> (batch b=c//2, query-half h=c%2). Per core:
QT~ (2048-query half) SBUF-resident after a single projection pass;
flash-style m-outer attention streaming X^T chunks with per-chunk softmax
stats and a deferred combine. Matmuls run as float32r (fp32, 11-bit
mantissa kept, fp32 PSUM accumulation) at full PE rate.

Engine balance in the attention phase (per 128x512 logits tile):
  PE:  8 accumulating matmuls
  DVE: negated max-reduce (feeds exp bias directly),
       fused (e * vbar) + accumulate via scalar_tensor_tensor
  Act: exp with accum_out (sum of exponentials for free)
"""

import numpy as np

import concourse.bacc as bacc
import concourse.mybir as mybir
import concourse.tile as tile
from concourse.bass_utils import run_bass_kernel_spmd

from contextlib import ExitStack

F32 = mybir.dt.float32
F32R = mybir.dt.float32r
BF16 = mybir.dt.bfloat16

D = 1024
DC = 8  # d chunks of 128
NQ = 2048  # queries per core
QT_N = 16  # q tiles of 128
M = 4096  # keys
MC = 8  # m chunks of 512

Exp = mybir.ActivationFunctionType.Exp
Alu = mybir.AluOpType
AxX = mybir.AxisListType.X


def build(n_cores=8):
    nc = bacc.Bacc("TRN2", target_bir_lowering=False, debug=False,
                   num_devices=n_cores)

    xt_d = nc.dram_tensor("xt", [DC, 128, M], F32R, kind="ExternalInput")
    xtq_d = nc.dram_tensor("xtq", [DC, 128, NQ], F32R, kind="ExternalInput")
    wqk_d = nc.dram_tensor("wqk", [DC, 128, D], F32R, kind="ExternalInput")
    wvb_d = nc.dram_tensor("wvb", [DC, 128, 128], F32R, kind="ExternalInput")
    # [128, 16]: column q holds the outputs of queries q*128..q*128+127;
    # the host transposes after download (single contiguous-line DMA here).
    out_d = nc.dram_tensor("out", [128, QT_N], F32, kind="ExternalOutput")

    with tile.TileContext(nc) as tc, ExitStack() as ctx:
        pw = ctx.enter_context(tc.tile_pool(name="pw", bufs=1))
        px = ctx.enter_context(tc.tile_pool(name="px", bufs=4))
        pqt = ctx.enter_context(tc.tile_pool(name="pqt", bufs=1))
        pvb = ctx.enter_context(tc.tile_pool(name="pvb", bufs=1))
        pe_ = ctx.enter_context(tc.tile_pool(name="pe", bufs=4))
        pj = ctx.enter_context(tc.tile_pool(name="pj", bufs=2))
        pst = ctx.enter_context(tc.tile_pool(name="pst", bufs=1))
        psm = ctx.enter_context(tc.tile_pool(name="psm", bufs=4))
        pps = ctx.enter_context(tc.tile_pool(name="pps", bufs=6, space="PSUM"))

        # ---- resident tiles ----
        qt_sb = [pqt.tile([128, NQ], F32R, name=f"qt{do}", tag=f"qt{do}")
                 for do in range(DC)]
        vbar_bc = pvb.tile([128, MC, 512], BF16, name="vbar", tag="vbar")

        # weights on the scalar-engine HWDGE queue: runs in parallel with the
        # xtq loads on the sync queue, so phase-1 matmuls start ~2x sooner
        wqk_t = [pw.tile([128, D], F32R, name=f"wqk{di}", tag=f"wqk{di}")
                 for di in range(DC)]
        for di in range(DC):
            nc.scalar.dma_start(wqk_t[di][:], wqk_d.ap()[di])
        wvb_t = [pw.tile([128, 128], F32R, name=f"wvb{di}", tag=f"wvb{di}")
                 for di in range(DC)]
        for di in range(DC):
            nc.scalar.dma_start(wvb_t[di][:], wvb_d.ap()[di])

        # ---- phase 1: QT~ = (X_half @ Wqk~)^T, kept SBUF-resident ----
        for n in range(NQ // 512):
            xq_t = [px.tile([128, 512], F32R, name=f"x{di}", tag=f"x{di}")
                    for di in range(DC)]
            for di in range(DC):
                nc.sync.dma_start(xq_t[di][:], xtq_d.ap()[di, :, n * 512:(n + 1) * 512])
            for do in range(DC):
                qtp = pps.tile([128, 512], F32, name="ps", tag="ps")
                for di in range(DC):
                    nc.tensor.matmul(qtp[:], wqk_t[di][:, do * 128:(do + 1) * 128],
                                     xq_t[di][:], start=(di == 0), stop=(di == DC - 1))
                nc.scalar.copy(qt_sb[do][:, n * 512:(n + 1) * 512], qtp[:])

        # ---- phase 2: attention, m-outer ----
        nmst = [pst.tile([128, MC], F32, name=f"m{q}", tag=f"m{q}") for q in range(QT_N)]
        dstk = [pst.tile([128, MC], F32, name=f"d{q}", tag=f"d{q}") for q in range(QT_N)]
        nstk = [pst.tile([128, MC], F32, name=f"n{q}", tag=f"n{q}") for q in range(QT_N)]

        for mi in range(MC):
            kt_t = [px.tile([128, 512], F32R, name=f"x{di}", tag=f"x{di}")
                    for di in range(DC)]
            for di in range(DC):
                nc.sync.dma_start(kt_t[di][:], xt_d.ap()[di, :, mi * 512:(mi + 1) * 512])
            # vbar chunk: all 128 partitions get identical rows
            vbp = pps.tile([128, 512], F32, name="ps", tag="ps")
            for di in range(DC):
                nc.tensor.matmul(vbp[:], wvb_t[di][:], kt_t[di][:],
                                 start=(di == 0), stop=(di == DC - 1))
            nc.scalar.copy(vbar_bc[:, mi, :], vbp[:])

            for q in range(QT_N):
                sp = pps.tile([128, 512], F32, name="ps", tag="ps")
                for di in range(DC):
                    nc.tensor.matmul(sp[:], qt_sb[di][:, q * 128:(q + 1) * 128],
                                     kt_t[di][:], start=(di == 0), stop=(di == DC - 1))
                nc.vector.tensor_reduce(nmst[q][:, mi:mi + 1], sp[:], axis=AxX,
                                        op=Alu.max, negate=True)
                e_t = pe_.tile([128, 512], BF16, name="e", tag="e")
                nc.scalar.activation(e_t[:], sp[:], Exp,
                                     bias=nmst[q][:, mi:mi + 1], scale=1.0,
                                     accum_out=dstk[q][:, mi:mi + 1])
                junk = pj.tile([128, 512], BF16, name="j", tag="j")
                nc.vector.scalar_tensor_tensor(junk[:], e_t[:], 1.0,
                                               vbar_bc[:, mi, :],
                                               op0=Alu.mult, op1=Alu.mult,
                                               accum_out=nstk[q][:, mi:mi + 1])

        # ---- combine + output ----
        o_all = pst.tile([128, QT_N], F32, name="oall", tag="oall")
        for q in range(QT_N):
            gm = psm.tile([128, 1], F32, name="gm", tag="gm")
            nc.vector.tensor_reduce(gm[:], nmst[q][:], axis=AxX, op=Alu.min)
            w8 = psm.tile([128, MC], F32, name="w8", tag="w8")
            nc.scalar.activation(w8[:], nmst[q][:], Exp, bias=gm[:], scale=-1.0)
            tn = psm.tile([128, MC], F32, name="tn", tag="tn")
            num = psm.tile([128, 1], F32, name="num", tag="num")
            nc.vector.scalar_tensor_tensor(tn[:], w8[:], 1.0, nstk[q][:],
                                           op0=Alu.mult, op1=Alu.mult,
                                           accum_out=num[:])
            td = psm.tile([128, MC], F32, name="td", tag="td")
            den = psm.tile([128, 1], F32, name="den", tag="den")
            nc.vector.scalar_tensor_tensor(td[:], w8[:], 1.0, dstk[q][:],
                                           op0=Alu.mult, op1=Alu.mult,
                                           accum_out=den[:])
            rec = psm.tile([128, 1], F32, name="rec", tag="rec")
            nc.vector.reciprocal(rec[:], den[:])
            nc.vector.tensor_tensor(o_all[:, q:q + 1], num[:], rec[:], op=Alu.mult)
            if q == QT_N // 2 - 1:
                nc.sync.dma_start(out_d.ap()[:, :QT_N // 2], o_all[:, :QT_N // 2])
        nc.sync.dma_start(out_d.ap()[:, QT_N // 2:], o_all[:, QT_N // 2:])

    nc.compile()
    return nc


def r32r(x):
    """Round fp32 -> fp32r (keep 11 mantissa bits, round-to-nearest-even)."""
    u = np.ascontiguousarray(x, dtype=np.float32).view(np.uint32)
    low = u & np.uint32(0xFFF)
    add = np.where((low > 0x800) | ((low == 0x800) & (((u >> np.uint32(12)) & 1) > 0)),
                   np.uint32(0x1000), np.uint32(0))
    return ((u + add) & np.uint32(0xFFFFF000)).view(np.float32)


def make_in_maps(inputs, Wq, Wk, Wv):
    """inputs [4,4096,1024] f32; weights [1024,1024]. Returns 8 in_maps."""
    B = inputs.shape[0]
    SCALE = np.float32(1.0 / 32.0)
    wqk = (Wq.astype(np.float32) @ Wk.astype(np.float32).T) * SCALE
    wqk_r = r32r(wqk).reshape(DC, 128, D)
    wvbar = (Wv.astype(np.float32).sum(axis=1) * np.float32(1.0 / D))
    wvb_r = np.repeat(r32r(wvbar).reshape(DC, 128, 1), 128, axis=2)
    wvb_r = np.ascontiguousarray(wvb_r)
    in_maps = []
    for c in range(2 * B):
        b, h = divmod(c, 2)
        xt = r32r(np.ascontiguousarray(inputs[b].T))  # [1024, 4096]
        xtq = np.ascontiguousarray(xt[:, h * NQ:(h + 1) * NQ])
        in_maps.append({
            "xt": np.ascontiguousarray(xt.reshape(DC, 128, M)),
            "xtq": np.ascontiguousarray(xtq.reshape(DC, 128, NQ)),
            "wqk": wqk_r, "wvb": wvb_r,
        })
    return in_maps


def assemble(results, B=4):
    out = np.empty((B, M), dtype=np.float32)
    for c in range(2 * B):
        b, h = divmod(c, 2)
        # device emits [128, 16] with column q = queries q*128..q*128+127
        out[b, h * NQ:(h + 1) * NQ] = np.ascontiguousarray(results[c]["out"].T).reshape(NQ)
    return out


_NC_CACHE = {}


def _get_nc():
    if "nc" not in _NC_CACHE:
        _NC_CACHE["nc"] = build(8)
    return _NC_CACHE["nc"]


def kernel(inputs, Wq, Wk, Wv):
    inputs = np.asarray(inputs, dtype=np.float32)
    Wq = np.asarray(Wq, dtype=np.float32)
    Wk = np.asarray(Wk, dtype=np.float32)
    Wv = np.asarray(Wv, dtype=np.float32)
    nc = _get_nc()
    in_maps = make_in_maps(inputs, Wq, Wk, Wv)
    res = run_bass_kernel_spmd(nc, in_maps, core_ids=list(range(8)), trace=False)
    return assemble(res.results, B=inputs.shape[0])
